# revision 19
# baseline (speedup 1.0000x reference)
"""Multi-head attention Trainium2 Bass kernel (v3).

Problem: B=8, S=1024, E=768, H=12, DH=64 MHA with per-head Q/K/V projections
and output projection. Data-parallel over batch: one batch element per
NeuronCore (8 cores).

v3 key change vs v2: the whole kernel is a flat software pipeline over
(rep, head-pair) with a FIFO work queue. Attention consumption (attn
matmuls + normalize + transpose) of pair i and the output projection of
rep r are split into small units that are interleaved one-per-two-slots
into later pairs' production. This keeps the ACT engine (softmax exps —
the true HW bottleneck at ~100us/rep) streaming continuously across pair
and rep boundaries instead of idling ~29us/rep while the PE runs the
output projection + next rep's QKV serially.

Per-core dataflow (per head-pair):
  xT [E,S] bf16  <- DMA-transpose of x (double-buffered across reps,
     prefetched during the previous rep's pair 4)
  qT/kT = W.T @ xT + b per head-pair [128(d-pair), S] bf16
  scoresT[t,s] = k @ q.T per head, row-packed K=64 concurrent matmuls
  expST = exp(0.125*scoresT) (ACT, bf16; no max subtraction: scores ~ N(0,1))
  v [s, d-pair] (xT chunk stationary, Wv moving) into vOnes with ones
     columns; bias via host-replicated [128,128] table (emitted after
     scores so the first exp of a pair starts as early as possible)
  attn[sq, d|Z] = expST_chunk.T @ [v|1]  (ex stationary [t,sq], N=65;
     Z accumulates in column 64)
  a_sb = attn * (1/Z)  (DVE reciprocal + free-dim-broadcast tensor_tensor)
  catT[d-pair, s] <- PE transpose of a_sb in [128,128] blocks
  outT[eo, s] = Wo_chunk.T @ catT + bo (per (eo, ch) unit, interleaved)
"""
import sys

sys.path.insert(0, "/opt/trn_rl_repo")

from collections import deque

import numpy as np
import ml_dtypes
from contextlib import ExitStack

import concourse.bass as bass
import concourse.tile as tile
from concourse import bacc, mybir
from concourse.bass_utils import run_bass_kernel_spmd
from concourse.masks import make_identity

F32 = mybir.dt.float32
BF16 = mybir.dt.bfloat16
AF = mybir.ActivationFunctionType
BF = ml_dtypes.bfloat16

B, S, E, H, DH = 8, 1024, 768, 12, 64
NP_ = 6          # head pairs
ET = 6           # e tiles of 128
ST = 8           # s tiles of 128
NCORES = 8

_cache = {}


def _build_nc(reps=1, ablate=""):
    if ("nc", reps, ablate) in _cache:
        return _cache[("nc", reps, ablate)]
    nc = bacc.Bacc("TRN2", target_bir_lowering=False, debug=False,
                   num_devices=NCORES)

    x = nc.dram_tensor("x", [S, E], BF16, kind="ExternalInput").ap()
    wq = nc.dram_tensor("wq", [NP_, 128, ET, 128], BF16, kind="ExternalInput").ap()
    wk = nc.dram_tensor("wk", [NP_, 128, ET, 128], BF16, kind="ExternalInput").ap()
    wv = nc.dram_tensor("wv", [NP_, 128, ET, 128], BF16, kind="ExternalInput").ap()
    bq = nc.dram_tensor("bq", [NP_, 128, 1], F32, kind="ExternalInput").ap()
    bk = nc.dram_tensor("bk", [NP_, 128, 1], F32, kind="ExternalInput").ap()
    bvr = nc.dram_tensor("bvr", [128, NP_, 128], F32, kind="ExternalInput").ap()
    wo = nc.dram_tensor("wo", [128, ET * E], BF16, kind="ExternalInput").ap()
    boT = nc.dram_tensor("boT", [128, ET], F32, kind="ExternalInput").ap()
    out = nc.dram_tensor("out", [E, S], F32, kind="ExternalOutput").ap()

    pull_every = 2 if "p2" in ablate else 1

    with tile.TileContext(nc) as tc, ExitStack() as ctx:
        consts = ctx.enter_context(tc.tile_pool(name="consts", bufs=1))
        xtp = ctx.enter_context(tc.tile_pool(name="xtp", bufs=2))
        catp = ctx.enter_context(tc.tile_pool(name="catp", bufs=2))
        wpool = ctx.enter_context(tc.tile_pool(name="wpool", bufs=3))
        qkp = ctx.enter_context(tc.tile_pool(name="qkp", bufs=3))
        vop = ctx.enter_context(tc.tile_pool(name="vop", bufs=3))
        exq = ctx.enter_context(tc.tile_pool(name="exq", bufs=3))
        asb = ctx.enter_context(tc.tile_pool(name="asb", bufs=2))
        zrp = ctx.enter_context(tc.tile_pool(name="zrp", bufs=4))
        osb = ctx.enter_context(tc.tile_pool(name="osb", bufs=3))
        # PSUM: mmp 2 + scp 2x2 + atp 2 = 8 banks
        mmp = ctx.enter_context(tc.tile_pool(name="mmp", bufs=2, space="PSUM"))
        scp = ctx.enter_context(tc.tile_pool(name="scp", bufs=2, space="PSUM"))
        atp = ctx.enter_context(tc.tile_pool(name="atp", bufs=2, space="PSUM"))

        ident = consts.tile([128, 128], BF16, tag="ident")
        make_identity(nc, ident)
        boT_t = consts.tile([128, ET], F32, tag="boT")
        nc.sync.dma_start(boT_t, boT)
        wo_t = consts.tile([128, ET * E], BF16, tag="wo")
        nc.sync.dma_start(wo_t, wo)
        bvr_t = consts.tile([128, NP_, 128], F32, tag="bvr")
        nc.sync.dma_start(bvr_t, bvr)

        pending = deque()   # FIFO of (data_birth_index, generator) units

        def pull(cur_i):
            # Strict lag 2: only emit units whose input data was produced
            # >= 2 pairs ago — pair p's exps are still streaming on ACT
            # during produce(p+1), and an attn matmul emitted early would
            # stall the in-order PE on the exp semaphore.
            while pending:
                if pending[0][0] > cur_i - 2:
                    return False
                try:
                    next(pending[0][1])
                    return True
                except StopIteration:
                    pending.popleft()
            return False

        def force_drain(upto):
            # Guarantee units with data_birth <= upto are fully emitted:
            # bounds queue lag so tile-ring reuse (exq/vop bufs=3) never
            # depends on instructions emitted later (deadlock).
            while pending and pending[0][0] <= upto:
                for _ in pending[0][1]:
                    pass
                pending.popleft()

        xT_gen = {}

        def issue_xT(r):
            xT = [xtp.tile([128, S], BF16, tag=f"xT{et}", name=f"xT{et}")
                  for et in range(ET)]
            for et in range(ET):
                nc.sync.dma_start(
                    xT[et], x[:, et * 128:(et + 1) * 128], transpose=True)
            xT_gen[r] = xT

        catT_gen = {}

        def produce(i, r, p):
            """QKV + scores + exp for pair (r, p); pulls one queued work
            unit every `pull_every` emission slots."""
            xT = xT_gen[r]
            slot = [0]

            def tick():
                if slot[0] % pull_every == 0:
                    pull(i)
                slot[0] += 1

            wq_t = wpool.tile([128, ET, 128], BF16, tag="wq", name="wq_t")
            nc.sync.dma_start(wq_t, wq[p])
            wk_t = wpool.tile([128, ET, 128], BF16, tag="wk", name="wk_t")
            nc.sync.dma_start(wk_t, wk[p])
            wv_t = wpool.tile([128, ET, 128], BF16, tag="wv", name="wv_t")
            nc.sync.dma_start(wv_t, wv[p])
            bq_t = wpool.tile([128, 1], F32, tag="bq", name="bq_t")
            nc.sync.dma_start(bq_t, bq[p])
            bk_t = wpool.tile([128, 1], F32, tag="bk", name="bk_t")
            nc.sync.dma_start(bk_t, bk[p])

            qT = qkp.tile([128, S], BF16, tag="qT", name="qT")
            kT = qkp.tile([128, S], BF16, tag="kT", name="kT")
            # DIAGNOSTIC qk1 (breaks correctness): 1 of 6 accum steps
            net = 1 if "qk1" in ablate else ET
            for w_t, b_t, dst in ((wq_t, bq_t, qT), (wk_t, bk_t, kT)):
                for ch in range(2):
                    pp = mmp.tile([128, 512], F32, tag="mm", name="pp")
                    for et in range(net):
                        nc.tensor.matmul(
                            pp, w_t[:, et, :],
                            xT[et][:, ch * 512:(ch + 1) * 512],
                            start=(et == 0), stop=(et == net - 1),
                        )
                    nc.vector.tensor_scalar_add(
                        dst[:, ch * 512:(ch + 1) * 512], pp, b_t)
                    tick()

            # scores + exp per (t, head): emitted before the v projection so
            # the pair's first exp starts as early as possible after qT/kT.
            # scores: one N=1024 matmul per head (bf16 moving operand max is
            # 1024), the two heads' K=64 matmuls back-to-back on disjoint
            # row groups so they stream concurrently through the array.
            ex_ts = [exq.tile([128, ST, S], BF16, tag=f"ex{e}",
                              name=f"ex{e}") for e in range(2)]
            for t in range(ST):
                scs = [scp.tile([128, S], F32, tag="sc", name="sc")
                       for _ in range(2)]
                for ch in range(2):
                    for e in range(2):
                        r0 = 64 * e
                        nc.tensor.matmul(
                            scs[e][:, ch * 512:(ch + 1) * 512],
                            kT[r0:r0 + 64, t * 128:(t + 1) * 128],
                            qT[r0:r0 + 64, ch * 512:(ch + 1) * 512],
                            tile_position=(r0, 0),
                            start=True, stop=True,
                            skip_group_check=True,
                        )
                for e in range(2):
                    if "exsmall" in ablate:
                        # DIAGNOSTIC (breaks correctness): 1/8th ACT work
                        nc.scalar.activation(ex_ts[e][:, t, 0:128],
                                             scs[e][:, 0:128],
                                             AF.Exp, scale=0.125)
                    else:
                        nc.scalar.activation(ex_ts[e][:, t, :], scs[e],
                                             AF.Exp, scale=0.125)
                tick()

            # v in [s, d-pair] layout packed into vOnes with ones columns
            vo = vop.tile([128, ST, 130], BF16, tag="vo", name="vo")
            nc.gpsimd.memset(vo.rearrange("p t d -> p (t d)"), 1.0)
            bvp = bvr_t[:, p, :].rearrange("p (two d) -> p two d", two=2)
            for stp in range(4):
                pv = mmp.tile([128, 2, 128], F32, tag="mm", name="pv")
                for s2 in range(2):
                    st = stp * 2 + s2
                    for et in range(ET):
                        nc.tensor.matmul(
                            pv[:, s2, :],
                            xT[et][:, st * 128:(st + 1) * 128],
                            wv_t[:, et, :],
                            start=(et == 0), stop=(et == ET - 1),
                        )
                for s2 in range(2):
                    st = stp * 2 + s2
                    nc.vector.tensor_tensor(
                        out=vo[:, st, :].rearrange(
                            "p (two dd) -> p two dd", two=2)[:, :, 0:64],
                        in0=pv[:, s2, :].rearrange(
                            "p (two d) -> p two d", two=2),
                        in1=bvp,
                        op=mybir.AluOpType.add,
                    )
                tick()

            # prefetch next rep's xT during pair 4 (DMA engines are idle)
            if p == 4 and r + 1 < reps:
                issue_xT(r + 1)

            return (vo, ex_ts)

        def consume_gen(r, p, state):
            """Attention for pair (r, p): flipped attn (ex stationary
            [128(t), 128(sq)], [v|1] moving N=65; Z lands in column 64),
            per-partition normalize, PE transpose back to catT."""
            vo, ex_ts = state
            if p == 0:
                catT_gen[r] = [
                    catp.tile([128, S], BF16, tag=f"catT{j}", name=f"catT{j}")
                    for j in range(NP_)]
            catT = catT_gen[r]
            a_sb = asb.tile([128, ST, 128], BF16, tag="asb", name="asb")
            for e in range(2):
                ex_t = ex_ts[e]
                for sh in range(2):
                    ap_ = atp.tile([128, 4, 65], F32, tag="att", name="att")
                    for sq4 in range(4):
                        sq = sh * 4 + sq4
                        # DIAGNOSTIC noattn (breaks correctness): 1 accum
                        tsteps = 1 if "noattn" in ablate else ST
                        for t in range(tsteps):
                            nc.tensor.matmul(
                                ap_[:, sq4, :],
                                ex_t[:, t, sq * 128:(sq + 1) * 128],
                                vo[:, t, 65 * e:65 * e + 65],
                                start=(t == 0), stop=(t == tsteps - 1),
                            )
                        if sq4 == 1:
                            yield
                    with tc.high_priority(offset=150):
                        zr = zrp.tile([128, 4], F32, tag="zr", name="zr")
                        nc.vector.reciprocal(zr, ap_[:, :, 64])
                        nc.vector.tensor_tensor(
                            out=a_sb[:, sh * 4:(sh + 1) * 4,
                                     64 * e:64 * e + 64],
                            in0=ap_[:, :, 0:64],
                            in1=zr[:, :, None].broadcast_to([128, 4, 64]),
                            op=mybir.AluOpType.mult,
                        )
                    yield
            # transpose a_sb [sq, d-pair] -> catT [d-pair, s]
            for g in range(2):
                tp = atp.tile([128, 4, 128], BF16, tag="att", name="tpa")
                for k in range(4):
                    st = g * 4 + k
                    nc.tensor.matmul(
                        tp[:, k, :], a_sb[:, st, :], ident,
                        is_transpose=True, skip_group_check=True,
                        start=True, stop=True,
                    )
                nc.vector.tensor_copy(
                    catT[p][:, g * 512:(g + 1) * 512],
                    tp.rearrange("p t d -> p (t d)"))
                yield

        def outproj_gen(r):
            """Output projection for rep r as 12 (eo, ch) units."""
            catT = catT_gen[r]
            for eo in range(ET):
                pool, ptag = (mmp, "mm") if eo % 2 == 0 else (scp, "sc")
                for ch in range(2):
                    pp = pool.tile([128, 512], F32, tag=ptag, name="op")
                    for j in range(NP_):
                        w_sl = wo_t[:, j * E + eo * 128:j * E + eo * 128 + 128]
                        nc.tensor.matmul(
                            pp, w_sl,
                            catT[j][:, ch * 512:(ch + 1) * 512],
                            start=(j == 0), stop=(j == NP_ - 1),
                        )
                    o_sb = osb.tile([128, 512], F32, tag="ot", name="ot")
                    nc.vector.tensor_scalar_add(o_sb, pp,
                                                boT_t[:, eo:eo + 1])
                    nc.sync.dma_start(
                        out[eo * 128:(eo + 1) * 128,
                            ch * 512:(ch + 1) * 512], o_sb)
                    yield
            del catT_gen[r]

        # ---- flat pipeline over (rep, pair) ----
        issue_xT(0)
        stream = [(r, p) for r in range(reps) for p in range(NP_)]
        states = {}
        for i, (r, p) in enumerate(stream):
            force_drain(i - 3)
            states[i] = (r, p, produce(i, r, p))
            if i >= 1:
                pr, pp_, st = states.pop(i - 1)
                pending.append((i - 1, consume_gen(pr, pp_, st)))
            if p == 0 and r >= 1:
                pending.append((i - 1, outproj_gen(r - 1)))
        i_last = len(stream) - 1
        pr, pp_, st = states.pop(i_last)
        pending.append((i_last, consume_gen(pr, pp_, st)))
        pending.append((i_last, outproj_gen(reps - 1)))
        while pull(i_last + 3):
            pass

    nc.compile()
    _cache[("nc", reps, ablate)] = nc
    return nc


def _prep_weights(Wq, bq, Wk, bk, Wv, bv, Wo, bo):
    def pack_w(W):  # [12, 768, 64] -> [6, 128, 6, 128] bf16
        Wp = W.reshape(NP_, 2, E, DH).transpose(0, 2, 1, 3).reshape(NP_, E, 128)
        return np.ascontiguousarray(
            Wp.reshape(NP_, ET, 128, 128).transpose(0, 2, 1, 3)).astype(BF)

    def pack_b(b):  # [12, 64] -> [6, 128, 1] f32
        return np.ascontiguousarray(b.reshape(NP_, 128, 1)).astype(np.float32)

    return {
        "wq": pack_w(Wq), "wk": pack_w(Wk), "wv": pack_w(Wv),
        "bq": pack_b(bq), "bk": pack_b(bk),
        "bvr": np.ascontiguousarray(np.broadcast_to(
            bv.reshape(1, NP_, 128), (128, NP_, 128))).astype(np.float32),
        "wo": np.ascontiguousarray(
            Wo.reshape(ET, 128, E).transpose(1, 0, 2).reshape(128, ET * E)
        ).astype(BF),
        "boT": np.ascontiguousarray(
            bo.reshape(ET, 128).T).astype(np.float32),
    }


def kernel(hidden_state, Wq, bq, Wk, bk, Wv, bv, Wo, bo):
    hidden_state = np.asarray(hidden_state, dtype=np.float32)
    shared = _prep_weights(
        np.asarray(Wq, np.float32), np.asarray(bq, np.float32),
        np.asarray(Wk, np.float32), np.asarray(bk, np.float32),
        np.asarray(Wv, np.float32), np.asarray(bv, np.float32),
        np.asarray(Wo, np.float32), np.asarray(bo, np.float32))
    nc = _build_nc()
    in_maps = [
        {"x": np.ascontiguousarray(hidden_state[b]).astype(BF), **shared}
        for b in range(NCORES)
    ]
    res = run_bass_kernel_spmd(nc, in_maps, core_ids=list(range(NCORES)))
    return np.stack([np.ascontiguousarray(r["out"].T) for r in res.results],
                    axis=0)


# revision 23
# speedup vs baseline: 1.0146x; 1.0146x over previous
"""Multi-head attention Trainium2 Bass kernel (v3).

Problem: B=8, S=1024, E=768, H=12, DH=64 MHA with per-head Q/K/V projections
and output projection. Data-parallel over batch: one batch element per
NeuronCore (8 cores).

v3 key change vs v2: the whole kernel is a flat software pipeline over
(rep, head-pair) with a FIFO work queue. Attention consumption (attn
matmuls + normalize + transpose) of pair i and the output projection of
rep r are split into small units that are interleaved one-per-two-slots
into later pairs' production. This keeps the ACT engine (softmax exps —
the true HW bottleneck at ~100us/rep) streaming continuously across pair
and rep boundaries instead of idling ~29us/rep while the PE runs the
output projection + next rep's QKV serially.

Per-core dataflow (per head-pair):
  xT [E,S] bf16  <- DMA-transpose of x (double-buffered across reps,
     prefetched during the previous rep's pair 4)
  qT/kT = W.T @ xT + b per head-pair [128(d-pair), S] bf16
  scoresT[t,s] = k @ q.T per head, row-packed K=64 concurrent matmuls
  expST = exp(0.125*scoresT) (ACT, bf16; no max subtraction: scores ~ N(0,1))
  v [s, d-pair] (xT chunk stationary, Wv moving) into vOnes with ones
     columns; bias via host-replicated [128,128] table (emitted after
     scores so the first exp of a pair starts as early as possible)
  attn[sq, d|Z] = expST_chunk.T @ [v|1]  (ex stationary [t,sq], N=65;
     Z accumulates in column 64)
  a_sb = attn * (1/Z)  (DVE reciprocal + free-dim-broadcast tensor_tensor)
  catT[d-pair, s] <- PE transpose of a_sb in [128,128] blocks
  outT[eo, s] = Wo_chunk.T @ catT + bo (per (eo, ch) unit, interleaved)
"""
import sys

sys.path.insert(0, "/opt/trn_rl_repo")

from collections import deque

import numpy as np
import ml_dtypes
from contextlib import ExitStack

import concourse.bass as bass
import concourse.tile as tile
from concourse import bacc, mybir
from concourse.bass_utils import run_bass_kernel_spmd
from concourse.masks import make_identity

F32 = mybir.dt.float32
BF16 = mybir.dt.bfloat16
AF = mybir.ActivationFunctionType
BF = ml_dtypes.bfloat16

B, S, E, H, DH = 8, 1024, 768, 12, 64
NP_ = 6          # head pairs
ET = 6           # e tiles of 128
ST = 8           # s tiles of 128
NCORES = 8

_cache = {}


def _build_nc(reps=1, ablate=""):
    if ("nc", reps, ablate) in _cache:
        return _cache[("nc", reps, ablate)]
    nc = bacc.Bacc("TRN2", target_bir_lowering=False, debug=False,
                   num_devices=NCORES)

    x = nc.dram_tensor("x", [S, E], BF16, kind="ExternalInput").ap()
    wq = nc.dram_tensor("wq", [NP_, 128, ET, 128], BF16, kind="ExternalInput").ap()
    wk = nc.dram_tensor("wk", [NP_, 128, ET, 128], BF16, kind="ExternalInput").ap()
    wv = nc.dram_tensor("wv", [NP_, 128, ET, 128], BF16, kind="ExternalInput").ap()
    bq = nc.dram_tensor("bq", [NP_, 128, 1], F32, kind="ExternalInput").ap()
    bk = nc.dram_tensor("bk", [NP_, 128, 1], F32, kind="ExternalInput").ap()
    bvr = nc.dram_tensor("bvr", [128, NP_, 128], F32, kind="ExternalInput").ap()
    wo = nc.dram_tensor("wo", [128, ET * E], BF16, kind="ExternalInput").ap()
    boT = nc.dram_tensor("boT", [128, ET], F32, kind="ExternalInput").ap()
    out = nc.dram_tensor("out", [E, S], F32, kind="ExternalOutput").ap()


    with tile.TileContext(nc) as tc, ExitStack() as ctx:
        consts = ctx.enter_context(tc.tile_pool(name="consts", bufs=1))
        xtp = ctx.enter_context(tc.tile_pool(name="xtp", bufs=2))
        catp = ctx.enter_context(tc.tile_pool(name="catp", bufs=2))
        wpool = ctx.enter_context(tc.tile_pool(name="wpool", bufs=3))
        qkp = ctx.enter_context(tc.tile_pool(name="qkp", bufs=3))
        vop = ctx.enter_context(tc.tile_pool(name="vop", bufs=3))
        exq = ctx.enter_context(tc.tile_pool(name="exq", bufs=3))
        asb = ctx.enter_context(tc.tile_pool(name="asb", bufs=2))
        zrp = ctx.enter_context(tc.tile_pool(name="zrp", bufs=4))
        osb = ctx.enter_context(tc.tile_pool(name="osb", bufs=3))
        # PSUM: mmp 2 + scp 2x2 + atp 2 = 8 banks
        mmp = ctx.enter_context(tc.tile_pool(name="mmp", bufs=2, space="PSUM"))
        scp = ctx.enter_context(tc.tile_pool(name="scp", bufs=2, space="PSUM"))
        atp = ctx.enter_context(tc.tile_pool(name="atp", bufs=2, space="PSUM"))

        ident = consts.tile([128, 128], BF16, tag="ident")
        make_identity(nc, ident)
        boT_t = consts.tile([128, ET], F32, tag="boT")
        nc.sync.dma_start(boT_t, boT)
        wo_t = consts.tile([128, ET * E], BF16, tag="wo")
        nc.sync.dma_start(wo_t, wo)
        bvr_t = consts.tile([128, NP_, 128], F32, tag="bvr")
        nc.sync.dma_start(bvr_t, bvr)

        pending = deque()   # FIFO of (data_birth_index, generator) units

        def pull(cur_i):
            # Strict lag 2: only emit units whose input data was produced
            # >= 2 pairs ago — pair p's exps are still streaming on ACT
            # during produce(p+1), and an attn matmul emitted early would
            # stall the in-order PE on the exp semaphore.
            while pending:
                if pending[0][0] > cur_i - 2:
                    return False
                try:
                    next(pending[0][1])
                    return True
                except StopIteration:
                    pending.popleft()
            return False

        def force_drain(upto):
            # Guarantee units with data_birth <= upto are fully emitted:
            # bounds queue lag so tile-ring reuse (exq/vop bufs=3) never
            # depends on instructions emitted later (deadlock).
            while pending and pending[0][0] <= upto:
                for _ in pending[0][1]:
                    pass
                pending.popleft()

        xT_gen = {}

        def issue_xT(r):
            xT = [xtp.tile([128, S], BF16, tag=f"xT{et}", name=f"xT{et}")
                  for et in range(ET)]
            for et in range(ET):
                nc.sync.dma_start(
                    xT[et], x[:, et * 128:(et + 1) * 128], transpose=True)
            xT_gen[r] = xT

        catT_gen = {}

        def produce(i, r, p):
            """QKV + scores + exp for pair (r, p); pulls one queued work
            units into PE slack."""
            xT = xT_gen[r]

            def tick(n=1):
                for _ in range(n):
                    if not pull(i):
                        break

            wq_t = wpool.tile([128, ET, 128], BF16, tag="wq", name="wq_t")
            nc.sync.dma_start(wq_t, wq[p])
            wk_t = wpool.tile([128, ET, 128], BF16, tag="wk", name="wk_t")
            nc.sync.dma_start(wk_t, wk[p])
            wv_t = wpool.tile([128, ET, 128], BF16, tag="wv", name="wv_t")
            nc.sync.dma_start(wv_t, wv[p])
            bq_t = wpool.tile([128, 1], F32, tag="bq", name="bq_t")
            nc.sync.dma_start(bq_t, bq[p])
            bk_t = wpool.tile([128, 1], F32, tag="bk", name="bk_t")
            nc.sync.dma_start(bk_t, bk[p])

            qT = qkp.tile([128, S], BF16, tag="qT", name="qT")
            kT = qkp.tile([128, S], BF16, tag="kT", name="kT")
            # DIAGNOSTIC qk1 (breaks correctness): 1 of 6 accum steps
            net = 1 if "qk1" in ablate else ET
            for w_t, b_t, dst in ((wq_t, bq_t, qT), (wk_t, bk_t, kT)):
                for ch in range(2):
                    pp = mmp.tile([128, 512], F32, tag="mm", name="pp")
                    for et in range(net):
                        nc.tensor.matmul(
                            pp, w_t[:, et, :],
                            xT[et][:, ch * 512:(ch + 1) * 512],
                            start=(et == 0), stop=(et == net - 1),
                        )
                    nc.vector.tensor_scalar_add(
                        dst[:, ch * 512:(ch + 1) * 512], pp, b_t)

            # scores + exp per (t, head): emitted before the v projection so
            # the pair's first exp starts as early as possible after qT/kT.
            # scores: one N=1024 matmul per head (bf16 moving operand max is
            # 1024), the two heads' K=64 matmuls back-to-back on disjoint
            # row groups so they stream concurrently through the array.
            ex_ts = [exq.tile([128, ST, S], BF16, tag=f"ex{e}",
                              name=f"ex{e}") for e in range(2)]
            # The scores+exp section is ACT-paced (~2.3us per t-chunk vs
            # ~1us of score matmuls): pull two queued units per t to fill
            # the PE while ACT drains the scp ring.
            for t in range(ST):
                for e in range(2):
                    r0 = 64 * e
                    sc = scp.tile([128, S], F32, tag="sc", name="sc")
                    for ch in range(2):
                        nc.tensor.matmul(
                            sc[:, ch * 512:(ch + 1) * 512],
                            kT[r0:r0 + 64, t * 128:(t + 1) * 128],
                            qT[r0:r0 + 64, ch * 512:(ch + 1) * 512],
                            tile_position=(r0, 0),
                            start=True, stop=True,
                            skip_group_check=True,
                        )
                    if "exsmall" in ablate:
                        # DIAGNOSTIC (breaks correctness): 1/8th ACT work
                        nc.scalar.activation(ex_ts[e][:, t, 0:128],
                                             sc[:, 0:128], AF.Exp, scale=0.125)
                    else:
                        nc.scalar.activation(ex_ts[e][:, t, :], sc,
                                             AF.Exp, scale=0.125)
                    tick()
                tick()

            # v in [s, d-pair] layout packed into vOnes with ones columns
            vo = vop.tile([128, ST, 130], BF16, tag="vo", name="vo")
            nc.gpsimd.memset(vo.rearrange("p t d -> p (t d)"), 1.0)
            bvp = bvr_t[:, p, :].rearrange("p (two d) -> p two d", two=2)
            for stp in range(4):
                pv = mmp.tile([128, 2, 128], F32, tag="mm", name="pv")
                for s2 in range(2):
                    st = stp * 2 + s2
                    for et in range(ET):
                        nc.tensor.matmul(
                            pv[:, s2, :],
                            xT[et][:, st * 128:(st + 1) * 128],
                            wv_t[:, et, :],
                            start=(et == 0), stop=(et == ET - 1),
                        )
                for s2 in range(2):
                    st = stp * 2 + s2
                    nc.vector.tensor_tensor(
                        out=vo[:, st, :].rearrange(
                            "p (two dd) -> p two dd", two=2)[:, :, 0:64],
                        in0=pv[:, s2, :].rearrange(
                            "p (two d) -> p two d", two=2),
                        in1=bvp,
                        op=mybir.AluOpType.add,
                    )
                tick()

            # prefetch next rep's xT during pair 4 (DMA engines are idle)
            if p == 4 and r + 1 < reps:
                issue_xT(r + 1)

            return (vo, ex_ts)

        def consume_gen(r, p, state):
            """Attention for pair (r, p): flipped attn (ex stationary
            [128(t), 128(sq)], [v|1] moving N=65; Z lands in column 64),
            per-partition normalize, PE transpose back to catT."""
            vo, ex_ts = state
            if p == 0:
                catT_gen[r] = [
                    catp.tile([128, S], BF16, tag=f"catT{j}", name=f"catT{j}")
                    for j in range(NP_)]
            catT = catT_gen[r]
            a_sb = asb.tile([128, ST, 128], BF16, tag="asb", name="asb")
            for e in range(2):
                ex_t = ex_ts[e]
                for sh in range(2):
                    ap_ = atp.tile([128, 4, 65], F32, tag="att", name="att")
                    for sq4 in range(4):
                        sq = sh * 4 + sq4
                        # DIAGNOSTIC noattn (breaks correctness): 1 accum
                        tsteps = 1 if "noattn" in ablate else ST
                        for t in range(tsteps):
                            nc.tensor.matmul(
                                ap_[:, sq4, :],
                                ex_t[:, t, sq * 128:(sq + 1) * 128],
                                vo[:, t, 65 * e:65 * e + 65],
                                start=(t == 0), stop=(t == tsteps - 1),
                            )
                        if sq4 == 1:
                            yield
                    with tc.high_priority(offset=150):
                        zr = zrp.tile([128, 4], F32, tag="zr", name="zr")
                        nc.vector.reciprocal(zr, ap_[:, :, 64])
                        nc.vector.tensor_tensor(
                            out=a_sb[:, sh * 4:(sh + 1) * 4,
                                     64 * e:64 * e + 64],
                            in0=ap_[:, :, 0:64],
                            in1=zr[:, :, None].broadcast_to([128, 4, 64]),
                            op=mybir.AluOpType.mult,
                        )
                    yield
            # transpose a_sb [sq, d-pair] -> catT [d-pair, s]
            for g in range(2):
                tp = atp.tile([128, 4, 128], BF16, tag="att", name="tpa")
                for k in range(4):
                    st = g * 4 + k
                    nc.tensor.matmul(
                        tp[:, k, :], a_sb[:, st, :], ident,
                        is_transpose=True, skip_group_check=True,
                        start=True, stop=True,
                    )
                nc.vector.tensor_copy(
                    catT[p][:, g * 512:(g + 1) * 512],
                    tp.rearrange("p t d -> p (t d)"))
                yield

        def outproj_gen(r):
            """Output projection for rep r as 12 (eo, ch) units."""
            catT = catT_gen[r]
            for eo in range(ET):
                pool, ptag = (mmp, "mm") if eo % 2 == 0 else (scp, "sc")
                for ch in range(2):
                    pp = pool.tile([128, 512], F32, tag=ptag, name="op")
                    for j in range(NP_):
                        w_sl = wo_t[:, j * E + eo * 128:j * E + eo * 128 + 128]
                        nc.tensor.matmul(
                            pp, w_sl,
                            catT[j][:, ch * 512:(ch + 1) * 512],
                            start=(j == 0), stop=(j == NP_ - 1),
                        )
                    o_sb = osb.tile([128, 512], F32, tag="ot", name="ot")
                    nc.vector.tensor_scalar_add(o_sb, pp,
                                                boT_t[:, eo:eo + 1])
                    nc.sync.dma_start(
                        out[eo * 128:(eo + 1) * 128,
                            ch * 512:(ch + 1) * 512], o_sb)
                    yield
            del catT_gen[r]

        # ---- flat pipeline over (rep, pair) ----
        issue_xT(0)
        stream = [(r, p) for r in range(reps) for p in range(NP_)]
        states = {}
        for i, (r, p) in enumerate(stream):
            force_drain(i - 3)
            states[i] = (r, p, produce(i, r, p))
            if i >= 1:
                pr, pp_, st = states.pop(i - 1)
                pending.append((i - 1, consume_gen(pr, pp_, st)))
            if p == 0 and r >= 1:
                pending.append((i - 1, outproj_gen(r - 1)))
        i_last = len(stream) - 1
        pr, pp_, st = states.pop(i_last)
        pending.append((i_last, consume_gen(pr, pp_, st)))
        pending.append((i_last, outproj_gen(reps - 1)))
        while pull(i_last + 3):
            pass

    nc.compile()
    _cache[("nc", reps, ablate)] = nc
    return nc


def _prep_weights(Wq, bq, Wk, bk, Wv, bv, Wo, bo):
    def pack_w(W):  # [12, 768, 64] -> [6, 128, 6, 128] bf16
        Wp = W.reshape(NP_, 2, E, DH).transpose(0, 2, 1, 3).reshape(NP_, E, 128)
        return np.ascontiguousarray(
            Wp.reshape(NP_, ET, 128, 128).transpose(0, 2, 1, 3)).astype(BF)

    def pack_b(b):  # [12, 64] -> [6, 128, 1] f32
        return np.ascontiguousarray(b.reshape(NP_, 128, 1)).astype(np.float32)

    return {
        "wq": pack_w(Wq), "wk": pack_w(Wk), "wv": pack_w(Wv),
        "bq": pack_b(bq), "bk": pack_b(bk),
        "bvr": np.ascontiguousarray(np.broadcast_to(
            bv.reshape(1, NP_, 128), (128, NP_, 128))).astype(np.float32),
        "wo": np.ascontiguousarray(
            Wo.reshape(ET, 128, E).transpose(1, 0, 2).reshape(128, ET * E)
        ).astype(BF),
        "boT": np.ascontiguousarray(
            bo.reshape(ET, 128).T).astype(np.float32),
    }


def kernel(hidden_state, Wq, bq, Wk, bk, Wv, bv, Wo, bo):
    hidden_state = np.asarray(hidden_state, dtype=np.float32)
    shared = _prep_weights(
        np.asarray(Wq, np.float32), np.asarray(bq, np.float32),
        np.asarray(Wk, np.float32), np.asarray(bk, np.float32),
        np.asarray(Wv, np.float32), np.asarray(bv, np.float32),
        np.asarray(Wo, np.float32), np.asarray(bo, np.float32))
    nc = _build_nc()
    in_maps = [
        {"x": np.ascontiguousarray(hidden_state[b]).astype(BF), **shared}
        for b in range(NCORES)
    ]
    res = run_bass_kernel_spmd(nc, in_maps, core_ids=list(range(NCORES)))
    return np.stack([np.ascontiguousarray(r["out"].T) for r in res.results],
                    axis=0)
